# Initial kernel scaffold
#
"""Bass/Trainium2 kernel for nn_EnhancedIterativePredictiveLayer.

Strategy
--------
Data-parallel over batch: B=32 samples -> 4 per core x 8 cores, all params
replicated (packed into two DRAM blobs: wpd [128,KP] partition-dim params,
wrow [1,KR] row params -> 3 inputs total, 2 const DMAs).  Per sample, all
state is kept in SBUF in transposed layout [D(partitions), T(free)] so every
matmul contracts over the partition dim with weights stationary:

  genT:    hT = relu(w1^T @ beliefT + b1);  predT = w2^T @ hT + b2
  qT     = q_w^T @ predT + q_b;   kT = k_w^T @ xT   (kT, v, cxT hoisted;
           k_b dropped: it shifts scores by a per-t constant the softmax
           over s cancels exactly)
  scoresT[s,t] = kT[:,s_tile]^T @ qT   (softmax over s = partition dim)
  exp with no max-subtraction (|scores| <= ~60, safe in fp32)
  attT_un[d,t] = sum_s v[s,d]*expT[s,t];  denom[t] = ones^T @ expT (PSUM acc)
  conv matmul is [conv_w1 | gate_w_pred]: its row CH is the gate's pred part
  gate   = sigmoid(row_CH + (gwa^T @ attT_un)/denom + gb)
  update = beliefT*(1-lr_m) + C1*predT + C2*attT_un, C1/C2 broadcast to
           [128,chunk] via K=1 ones-outer-product matmuls

beliefT aliases xT (x is only consumed via the hoisted kT/v/cxT and the
controller, all computed before the first update), so the update runs in
place.  x loads are one 3D-AP DMA per sample, prefetched one pair ahead;
writeback stages transposed belief in the dead pred tiles so the x staging
tile frees early.

The controller runs on device; iterations 0/1 are peeled (controller's
gen(x) is reused as iteration 0's pred).  One dynamic For_i loop per sample
PAIR covers iterations 2..max(counts)-1 (zero trips on typical data), with
per-sample masking via `completed`; the count exit is tracked as
rem = counts-(i+1), decremented each body, completed |= rem<0.5 -- exactly
the reference's `done = (i+1)>=counts | converged` masking.

Engine assignment (HW-validated): PE matmuls/transposes; Activation does
exp/relu and PSUM->SBUF copies; DVE does biases, gate rows, and the update;
the GpSimd (Pool) engine is deliberately UNUSED -- real gpsimd ops cost
~2-3us each (launch overhead), ~7x the CoreSim cost model, and GPSIMD
cannot access PSUM at all.  All matmuls use float32r (full PE rate at
moving-dim >= 256); every fp32r-consumed location is written through an
f32r-rounding view as the walrus verifier requires.
"""

import os

import numpy as np
from contextlib import ExitStack

import concourse.bass as bass
import concourse.bacc as bacc
import concourse.tile as tile
import concourse.mybir as mybir
from concourse.bass_utils import run_bass_kernel_spmd

F32 = mybir.dt.float32
F32R = mybir.dt.float32r
I32 = mybir.dt.int32
AF = mybir.ActivationFunctionType
ALU = mybir.AluOpType
AX = mybir.AxisListType

N_CORES = 8
B, T, D = 32, 1024, 256
BPC = B // N_CORES            # samples per core
HID = 256                     # gen hidden
ATT_H = 128                   # attention hidden (exactly one partition tile)
CH = 64                       # conv hidden
ITER_H = 64
MIN_ITER, MAX_ITER = 2, 8
TCH = 512                     # t-chunk (matmul moving dim)
NT = T // TCH                 # 2 chunks
NS = T // 128                 # 8 key tiles
ND = D // 128                 # 2 partition tiles of D
# packed-param blob layout (see _load_consts / make_in_maps)
PER_C = HID + D + ATT_H + ATT_H + D + (CH + 1) + 1
KP_RND = ND * PER_C + 1       # f32r-rounded region (stationary operands)
KP = KP_RND + 2 * ND + 2 + 1 + 128   # + b1,b2,qb,kb,cb1,ident
KR = D + 3 * ITER_H + (MAX_ITER - MIN_ITER) + 4


def _r(ap):
    return ap.bitcast(F32R)


class _Emit:
    def __init__(self, ctx, tc, dram):
        self.ctx = ctx
        self.tc = tc
        self.nc = tc.nc
        self.d = dram
        p = lambda name, bufs, space="SBUF": ctx.enter_context(
            tc.tile_pool(name=name, bufs=bufs, space=space)
        )
        self.const = p("const", 1)
        self.xio = p("xio", 1)           # dedicated DMA-in / DMA-out staging
        self.state = p("state", 1)
        self.work = p("work", 3)
        self.expp = p("expp", 12)
        self.rows = p("rows", 1)
        self.psG = p("psG", 2, "PSUM")   # gen / q / kT / v matmuls
        self.psS = p("psS", 2, "PSUM")   # scores + transposes
        self.psA = p("psA", 2, "PSUM")   # attended accumulators + C1/C2
        self.psD = p("psD", 1, "PSUM")   # softmax denominator
        self.psR = p("psR", 1, "PSUM")   # M=1 rows: gate/conv2/sumsq
        self.xin_tiles = {}
        self.pending_x = {}
        self._load_consts()

    # ---------------- constants / weights ----------------
    # All partition-dim params live in one [128, KP] blob (single DMA), all
    # row params in one [1, KR] blob.  Column layout must match make_in_maps.
    def _load_consts(self):
        nc, d = self.nc, self.d
        t = lambda shape, tag, dt=F32: self.const.tile(shape, dt, tag=tag, name=tag)
        dma = nc.sync.dma_start

        praw = t([128, KP], "wpd_raw")
        dma(praw[:], d["wpd"])
        rraw = t([1, KR], "wrow_raw")
        dma(rraw[:], d["wrow"])
        # one rounded copy of the matmul-stationary region
        prnd = t([128, KP_RND], "wpd_rnd")
        nc.vector.tensor_copy(_r(prnd[:]), praw[:, 0:KP_RND])

        def rv(off, w):  # rounded view
            return prnd[:, off:off + w]

        per_c = HID + D + ATT_H + ATT_H + D + (CH + 1) + 1
        self.w1, self.w2, self.qw, self.kw, self.vw, self.cw1g, self.gwa = \
            [], [], [], [], [], [], []
        for c in range(ND):
            o = c * per_c
            self.w1.append(rv(o, HID)); o += HID
            self.w2.append(rv(o, D)); o += D
            self.qw.append(rv(o, ATT_H)); o += ATT_H
            self.kw.append(rv(o, ATT_H)); o += ATT_H
            self.vw.append(rv(o, D)); o += D
            self.cw1g.append(rv(o, CH + 1)); o += CH + 1
            self.gwa.append(rv(o, 1)); o += 1
        self.cw2 = prnd[0:CH, ND * per_c:ND * per_c + 1]

        uo = KP_RND
        def uv(w, rows=128):  # unrounded view
            nonlocal uo
            ap = praw[0:rows, uo:uo + w]
            uo += w
            return ap
        self.b1 = [uv(1) for _ in range(ND)]
        self.b2 = [uv(1) for _ in range(ND)]
        self.qb = uv(1, ATT_H)
        self.kb = uv(1, ATT_H)
        self.cb1 = uv(1, CH)
        self.ident = uv(128)

        ro = 0
        def rview(w):
            nonlocal ro
            ap = rraw[:, ro:ro + w]
            ro += w
            return ap
        vbrow = rview(D)
        self.iw1 = rview(ITER_H)
        self.iw2 = rview(ITER_H)
        self.ib1 = rview(ITER_H)
        self.ladder = rview(MAX_ITER - MIN_ITER)
        self.gate_b = rview(1)
        self.conv_b2 = rview(1)
        self.iter_b2 = rview(1)
        self.lr = rview(1)

        ones0 = t([128, 1], "ones0")
        nc.vector.memset(ones0[:], 1.0)
        self.ones = t([128, 1], "ones", F32R)
        nc.vector.tensor_copy(self.ones[:], _r(ones0[:]))
        self.ones_row0 = t([1, 128], "ones_row0")
        nc.vector.memset(self.ones_row0[:], 1.0)
        self.ones_row = t([1, 128], "ones_row", F32R)
        nc.vector.tensor_copy(self.ones_row[:], _r(self.ones_row0[:]))
        vbrow_r = t([1, D], "vbrow_r")
        nc.vector.tensor_copy(_r(vbrow_r[:]), vbrow)
        vb_ps = self.psG.tile([128, D], F32, tag="gen", name="vb_ps")
        nc.tensor.matmul(vb_ps[:], self.ones_row[:], _r(vbrow_r[:]),
                         start=True, stop=True)
        self.VB = t([128, D], "VB")
        nc.vector.tensor_copy(self.VB[:], vb_ps[:])

        self.neg_lr = t([1, 1], "neg_lr")
        nc.vector.tensor_scalar_mul(self.neg_lr[:], self.lr, -1.0)
        self.neg_gate_b = t([1, 1], "neg_gate_b")
        nc.vector.tensor_scalar_mul(self.neg_gate_b[:], self.gate_b, -1.0)
        self.neg_conv_b2 = t([1, 1], "neg_conv_b2")
        nc.vector.tensor_scalar_mul(self.neg_conv_b2[:], self.conv_b2, -1.0)
        self.neg_iter_b2 = t([1, 1], "neg_iter_b2")
        nc.vector.tensor_scalar_mul(self.neg_iter_b2[:], self.iter_b2, -1.0)
        # coefA/coefB: per-iteration [lr_eff, -lr_eff, 1-lr_eff] =
        # coefA*completed + coefB with coefA=[-lr,lr,lr], coefB=[lr,-lr,1-lr]
        self.coefA = t([1, 3], "coefA")
        self.coefB = t([1, 3], "coefB")
        nc.vector.tensor_copy(self.coefA[:, 0:1], self.neg_lr[:])
        nc.vector.tensor_copy(self.coefA[:, 1:2], self.lr)
        nc.vector.tensor_copy(self.coefA[:, 2:3], self.lr)
        nc.vector.tensor_copy(self.coefB[:, 0:1], self.lr)
        nc.vector.tensor_copy(self.coefB[:, 1:2], self.neg_lr[:])
        nc.vector.tensor_scalar(self.coefB[:, 2:3], self.lr, -1.0, 1.0,
                                ALU.mult, ALU.add)

    # ---------------- helpers ----------------
    def gen(self, src, pred, par=0):
        """pred[c][:, :] = (relu(src @ w1 + b1) @ w2 + b2)^T, src/pred: 2x[128,T]."""
        nc = self.nc
        hT = [self.work.tile([128, T], F32, tag=f"hT{par}_{c}", name=f"hT{par}_{c}",
                             bufs=1) for c in range(ND)]
        for tch in range(NT):
            sl = bass.ts(tch, TCH)
            for hc in range(ND):
                ps = self.psG.tile([128, TCH], F32, tag="gen", name="gen")
                for c in range(ND):
                    nc.tensor.matmul(
                        ps[:], _r(self.w1[c][:, bass.ts(hc, 128)]), _r(src[c][:, sl]),
                        start=(c == 0), stop=(c == ND - 1),
                    )
                nc.scalar.activation(_r(hT[hc][:, sl]), ps[:], AF.Relu, bias=self.b1[hc])
        for tch in range(NT):
            sl = bass.ts(tch, TCH)
            for dc in range(ND):
                ps = self.psG.tile([128, TCH], F32, tag="gen", name="gen")
                for c in range(ND):
                    nc.tensor.matmul(
                        ps[:], _r(self.w2[c][:, bass.ts(dc, 128)]), _r(hT[c][:, sl]),
                        start=(c == 0), stop=(c == ND - 1),
                    )
                nc.scalar.activation(_r(pred[dc][:, sl]), ps[:], AF.Identity,
                                     bias=self.b2[dc])

    # ---------------- per-sample program ----------------
    def prefetch_x(self, s):
        """Issue the x DMA for sample s as early as its staging tile is free
        (tag cycles s%2, so the WAR is on the prior-but-one sample's
        transposes, which finish early).  prep() consumes the pending tile."""
        if s in self.pending_x:
            return self.pending_x.pop(s)
        nc, d = self.nc, self.d
        xin = self.xin_tiles.get(s % 2)
        if xin is None:
            xin = self.xio.tile([128, NS, D], F32, tag=f"xio{s % 2}",
                                name=f"xio{s % 2}")
            self.xin_tiles[s % 2] = xin
        nc.sync.dma_start(xin[:], d["x"][s].rearrange("n p d -> p n d"))
        self.pending_x[s] = xin
        return xin

    def prep(self, s):
        nc, d = self.nc, self.d
        st = lambda shape, tag, dt=F32: self.state.tile(shape, dt, tag=tag, name=tag)

        # ---- load x with one DMA into a 3D staging tile [128, NS, D];
        # usually already issued early by prefetch_x during the prior pair ----
        xin = self.prefetch_x(s)

        # ---- transpose to xT [D, T]; xT doubles as beliefT (updated in place)
        xT = [st([128, T], f"xT{s % 2}_{c}") for c in range(ND)]
        for g in range(2):
            for c in range(ND):
                ps = self.psS.tile([128, 512], F32, tag="sc", name="tx")
                for j in range(4):
                    nc.tensor.transpose(
                        ps[:, bass.ts(j, 128)], xin[:, g * 4 + j, bass.ts(c, 128)],
                        self.ident
                    )
                nc.scalar.copy(_r(xT[c][:, g * 512:(g + 1) * 512]), ps[:])

        # ---- hoisted: kT, v, cxT ----
        kT = st([ATT_H, T], f"kT{s % 2}")
        for tch in range(NT):
            sl = bass.ts(tch, TCH)
            ps = self.psG.tile([128, TCH], F32, tag="gen", name="gen")
            for c in range(ND):
                nc.tensor.matmul(ps[:], _r(self.kw[c]), _r(xT[c][:, sl]),
                                 start=(c == 0), stop=(c == ND - 1))
            # k_b is dropped: it shifts score[s,t] by a per-t constant, which
            # the softmax over s cancels exactly.
            nc.scalar.copy(_r(kT[:, sl]), ps[:])
        v = st([128, NS * D], f"v{s % 2}")
        for n in range(NS):
            ps = self.psG.tile([128, D], F32, tag="gen", name="gen")
            for c in range(ND):
                nc.tensor.matmul(ps[:], _r(xT[c][:, bass.ts(n, 128)]), _r(self.vw[c]),
                                 start=(c == 0), stop=(c == ND - 1))
            nc.vector.tensor_add(_r(v[:, bass.ts(n, D)]), ps[:], self.VB[:])
        cxT = st([CH, T], f"cxT{s % 2}")
        for tch in range(NT):
            sl = bass.ts(tch, TCH)
            ps = self.psR.tile([CH + 1, TCH], F32, tag="g", name="g")
            for c in range(ND):
                nc.tensor.matmul(ps[:], _r(self.cw1g[c]), _r(xT[c][:, sl]),
                                 start=(c == 0), stop=(c == ND - 1))
            nc.scalar.copy(cxT[:, sl], ps[0:CH, :])
        completed = self.rows.tile([1, 1], F32, tag=f"completed{s % 2}",
                                   name=f"completed{s % 2}")
        nc.vector.memset(completed[:], 0.0)

        # ---- controller: gen(x), max row-norm of error, tiny MLP -> counts ----
        pred = [st([128, T], f"pred{s % 2}_{c}") for c in range(ND)]
        self.gen(xT, pred, par=s % 2)
        mep = self.rows.tile([1, NT], F32, tag="mep", name="mep")
        for tch in range(NT):
            sl = bass.ts(tch, TCH)
            ps = self.psR.tile([1, TCH], F32, tag="g", name="g")
            for c in range(ND):
                err0 = self.work.tile([128, TCH], F32, tag="err0", name="err0", bufs=1)
                nc.vector.tensor_sub(err0[:], xT[c][:, sl], pred[c][:, sl])
                err = self.work.tile([128, TCH], F32, tag="err", name="err", bufs=1)
                nc.scalar.activation(_r(err[:]), err0[:], AF.Square)
                nc.tensor.matmul(ps[:], self.ones[:], _r(err[:]),
                                 start=(c == 0), stop=(c == ND - 1))
            nc.vector.reduce_max(mep[:, tch:tch + 1], ps[:], axis=AX.X)
        me = self.rows.tile([1, 1], F32, tag="me", name="me")
        nc.vector.reduce_max(me[:], mep[:], axis=AX.X)
        nc.scalar.activation(me[:], me[:], AF.Ln)
        nc.scalar.activation(me[:], me[:], AF.Exp, scale=0.5)
        hrow = self.rows.tile([1, ITER_H], F32, tag="hrow", name="hrow")
        nc.vector.tensor_scalar_mul(hrow[:], self.iw1, me[:])
        nc.vector.tensor_add(hrow[:], hrow[:], self.ib1)
        nc.scalar.activation(hrow[:], hrow[:], AF.Relu)
        nc.vector.tensor_mul(hrow[:], hrow[:], self.iw2)
        inten = self.rows.tile([1, 1], F32, tag="inten", name="inten")
        nc.vector.reduce_sum(inten[:], hrow[:], axis=AX.X)
        nc.scalar.activation(inten[:], inten[:], AF.Exp, scale=-1.0,
                             bias=self.neg_iter_b2[:])
        nc.vector.tensor_scalar_add(inten[:], inten[:], 1.0)
        nc.vector.reciprocal_approx_fast(inten[:], inten[:])
        nc.vector.tensor_scalar(inten[:], inten[:], float(MAX_ITER - MIN_ITER),
                                float(MIN_ITER), ALU.mult, ALU.add)
        lad = self.rows.tile([1, MAX_ITER - MIN_ITER], F32, tag="lad", name="lad")
        nc.vector.tensor_scalar(lad[:], self.ladder, inten[:], None, ALU.is_lt)
        cntf = self.rows.tile([1, 1], F32, tag=f"cntf{s % 2}", name=f"cntf{s % 2}")
        nc.vector.reduce_sum(cntf[:], lad[:], axis=AX.X)
        nc.vector.tensor_scalar_add(cntf[:], cntf[:], float(MIN_ITER))
        # rem = counts - (i+1) tracked incrementally; completed |= rem <= 0
        # at the end of each body exactly mirrors the reference's
        # `done = (i+1) >= counts` without needing the loop index.
        rem = self.rows.tile([1, 1], F32, tag=f"rem{s % 2}", name=f"rem{s % 2}")
        nc.vector.tensor_scalar_add(rem[:], cntf[:], -1.0)
        return dict(s=s, xT=xT, kT=kT, v=v, cxT=cxT, pred=pred,
                    completed=completed, cntf=cntf, rem=rem)

    def run_body(self, st_, first):
        s = st_["s"]
        self.body(st_["kT"], st_["v"], st_["cxT"], st_["xT"], st_["pred"],
                  st_["completed"], first=first, par=s % 2,
                  rem=None if first else st_["rem"])

    def run_pair_loop(self, st0, st1):
        # One dynamic loop for both samples of a pair, masked per sample via
        # completed (count-exit folded in by the rem bookkeeping).  Covers
        # 2..max(counts)-1; zero trips when both counts == 2.
        nc = self.nc
        cmax = self.rows.tile([1, 1], F32, tag="cmax", name="cmax")
        nc.vector.tensor_max(cmax[:], st0["cntf"][:], st1["cntf"][:])
        cnti = self.rows.tile([1, 1], I32, tag=f"cnti{st0['s']}",
                              name=f"cnti{st0['s']}")
        nc.vector.tensor_copy(cnti[:], cmax[:])
        counts = nc.values_load(cnti[:], min_val=MIN_ITER, max_val=MAX_ITER,
                                skip_runtime_bounds_check=True)
        with self.tc.For_i(2, counts):
            self.run_body(st0, first=False)
            self.run_body(st1, first=False)

    def writeback(self, st_):
        nc, d = self.nc, self.d
        s, bel, pred = st_["s"], st_["xT"], st_["pred"]
        # transpose belief -> [T, D]; stage in the (dead) pred tiles so the
        # xio input tile is released right after prep and the next pair's x
        # DMA can prefetch during this pair's bodies.
        for np_ in range(NS // 2):
            n0 = 2 * np_
            ps = self.psS.tile([128, 2 * D], F32, tag="sc", name="txo")
            for j in range(2):
                for c in range(ND):
                    nc.tensor.transpose(
                        ps[:, j * D + c * 128: j * D + (c + 1) * 128],
                        bel[c][:, bass.ts(n0 + j, 128)], self.ident)
            stg = pred[n0 // (NS // ND)]
            off = (n0 % (NS // ND)) * D
            nc.scalar.copy(_r(stg[:, off:off + 2 * D]), ps[:])
            if n0 % 4 == 2:
                # one DMA per staged half-sample (4 n-blocks) instead of 8
                g = n0 // 4
                nc.sync.dma_start(
                    d["out"][s, 4 * g:4 * g + 4].rearrange("n p d -> p n d"),
                    stg[:, 0:4 * D].rearrange("p (n d) -> p n d", n=4))

    # ---------------- one iteration ----------------
    def body(self, kT, v, cxT, bel, pred, completed, first, par=0, rem=None):
        nc = self.nc
        if not first:
            self.gen(bel, pred, par=par)

        # coef = [lr_eff, -lr_eff, 1-lr_eff] in one fused op
        coef = self.rows.tile([1, 3], F32, tag=f"coef{par}", name=f"coef{par}")
        nc.vector.scalar_tensor_tensor(coef[:], self.coefA[:], completed[:],
                                       self.coefB[:], ALU.mult, ALU.add)
        lr_eff = coef[0:1, 0:1]
        nlr_eff = coef[0:1, 1:2]
        c3row = self.rows.tile([1, 128], F32, tag="c3row", name="c3row")
        nc.vector.tensor_scalar_mul(_r(c3row[:]), self.ones_row0[:], coef[0:1, 2:3])
        c3ps = self.psR.tile([128, 128], F32, tag="g", name="c3ps")
        nc.tensor.matmul(c3ps[:], self.ones_row[:], _r(c3row[:]), start=True, stop=True)
        c3col = self.rows.tile([128, 1], F32, tag="c3col", name="c3col")
        nc.vector.tensor_copy(c3col[:], c3ps[:, 0:1])

        acc = [self.rows.tile([1, 1], F32, tag=f"cacc{par}_{i}", name=f"cacc{par}_{i}")
               for i in range(NT)]
        if not hasattr(self, "gpred"):
            self.gpred = {}
        if par not in self.gpred:
            self.gpred[par] = self.rows.tile([1, T], F32, tag=f"gpred{par}",
                                             name=f"gpred{par}")
        for tch in range(NT):
            sl = bass.ts(tch, TCH)

            # qT chunk = q_w^T @ pred + q_b
            qT = self.work.tile([ATT_H, TCH], F32, tag="qT", name="qT", bufs=3)
            ps = self.psG.tile([128, TCH], F32, tag="gen", name="gen")
            for c in range(ND):
                nc.tensor.matmul(ps[:], _r(self.qw[c]), _r(pred[c][:, sl]),
                                 start=(c == 0), stop=(c == ND - 1))
            nc.scalar.activation(_r(qT[:]), ps[:], AF.Identity, bias=self.qb)

            # conv detector chunk on err = x - pred (cxT hoisted); row CH of
            # the folded matmul is the gate's pred part, read in place below.
            cps = self.psR.tile([CH + 1, TCH], F32, tag="g", name="g")
            for c in range(ND):
                nc.tensor.matmul(cps[:], _r(self.cw1g[c]), _r(pred[c][:, sl]),
                                 start=(c == 0), stop=(c == ND - 1))
            gpred = self.gpred[par]
            nc.vector.tensor_copy(gpred[:, sl], cps[CH:CH + 1, :])
            ch0 = self.work.tile([CH, TCH], F32, tag="ch0", name="ch0", bufs=1)
            nc.vector.tensor_sub(ch0[:], cxT[:, sl], cps[0:CH, :])
            convh = self.work.tile([CH, TCH], F32, tag="convh", name="convh",
                                   bufs=1)
            nc.scalar.activation(_r(convh[:]), ch0[:], AF.Relu, bias=self.cb1)
            ps2 = self.psR.tile([1, TCH], F32, tag="g", name="g2")
            nc.tensor.matmul(ps2[:], _r(self.cw2), _r(convh[:]),
                             start=True, stop=True)
            crow = self.rows.tile([1, TCH], F32, tag="crow", name="crow")
            nc.scalar.activation(crow[:], ps2[:], AF.Exp, scale=-1.0,
                                 bias=self.neg_conv_b2[:])
            nc.vector.tensor_scalar_add(crow[:], crow[:], 1.0)
            nc.vector.reciprocal_approx_fast(crow[:], crow[:])
            nc.vector.reduce_sum(acc[tch][:], crow[:], axis=AX.X)

            # scores + exp
            expt = []
            for n in range(NS):
                ps = self.psS.tile([128, TCH], F32, tag="sc", name="sc")
                nc.tensor.matmul(ps[:], _r(kT[:, bass.ts(n, 128)]), _r(qT[:]),
                                 start=True, stop=True)
                et = self.expp.tile([128, TCH], F32, tag="exp", name="exp")
                nc.scalar.activation(_r(et[:]), ps[:], AF.Exp)
                expt.append(et)

            # softmax denominator: Pool tree-add + partition all-reduce
            # (replaces 8 accumulating ones-matmuls on the PE)
            # attended accumulation + denominator (ones-matmul, PE)
            psat = [self.psA.tile([128, TCH], F32, tag="att", name="att")
                    for _ in range(ND)]
            psd = self.psD.tile([1, TCH], F32, tag="den", name="den")
            for n in range(NS):
                for c in range(ND):
                    nc.tensor.matmul(psat[c][:],
                                     _r(v[:, n * D + 128 * c: n * D + 128 * (c + 1)]),
                                     _r(expt[n][:]), start=(n == 0), stop=(n == NS - 1))
                nc.tensor.matmul(psd[:], self.ones[:], _r(expt[n][:]),
                                 start=(n == 0), stop=(n == NS - 1))
            recip = self.rows.tile([1, TCH], F32, tag="recip", name="recip")
            nc.vector.reciprocal_approx_fast(recip[:], psd[:])
            attT = []
            for c in range(ND):
                at = self.work.tile([128, TCH], F32, tag=f"attT{c}", name=f"attT{c}")
                nc.scalar.copy(_r(at[:]), psat[c][:])
                attT.append(at)

            # gate row (pred part = conv matmul row CH, read from PSUM)
            psg2 = self.psD.tile([1, TCH], F32, tag="den", name="gate2")
            for c in range(ND):
                nc.tensor.matmul(psg2[:], _r(self.gwa[c]), _r(attT[c][:]),
                                 start=(c == 0), stop=(c == ND - 1))
            grow = self.rows.tile([1, TCH], F32, tag="grow", name="grow")
            nc.vector.tensor_mul(grow[:], psg2[:], recip[:])
            gate = self.rows.tile([1, TCH], F32, tag="gate", name="gate")
            nc.vector.tensor_add(gate[:], gpred[:, sl], grow[:])
            nc.scalar.activation(gate[:], gate[:], AF.Exp, scale=-1.0,
                                 bias=self.neg_gate_b[:])
            nc.vector.tensor_scalar_add(gate[:], gate[:], 1.0)
            nc.vector.reciprocal_approx_fast(gate[:], gate[:])

            # coefficient rows + broadcast (Pool)
            c1r = self.rows.tile([1, TCH], F32, tag="c1r", name="c1r")
            nc.vector.tensor_scalar(_r(c1r[:]), gate[:], nlr_eff, lr_eff,
                                    ALU.mult, ALU.add)
            c2r = self.rows.tile([1, TCH], F32, tag="c2r", name="c2r")
            nc.vector.scalar_tensor_tensor(_r(c2r[:]), gate[:], lr_eff, recip[:],
                                           ALU.mult, ALU.mult)
            C1 = self.psA.tile([128, TCH], F32, tag="att", name="C1")
            nc.tensor.matmul(C1[:], self.ones_row[:], _r(c1r[:]), start=True, stop=True)
            C2 = self.psA.tile([128, TCH], F32, tag="att", name="C2")
            nc.tensor.matmul(C2[:], self.ones_row[:], _r(c2r[:]), start=True, stop=True)

            # belief <- belief*c3 + C1*pred + C2*attT_un   (in place)
            for c in range(ND):
                t0 = self.work.tile([128, TCH], F32, tag="upd0", name="upd0")
                nc.vector.tensor_mul(t0[:], pred[c][:, sl], C1[:])
                t1 = self.work.tile([128, TCH], F32, tag="upd1", name="upd1")
                nc.vector.tensor_mul(t1[:], attT[c][:], C2[:])
                nc.vector.tensor_add(t0[:], t0[:], t1[:])
                nc.vector.scalar_tensor_tensor(_r(bel[c][:, sl]), bel[c][:, sl],
                                               c3col[:], t0[:], ALU.mult, ALU.add)

        convf = self.rows.tile([1, 1], F32, tag=f"convf{par}", name=f"convf{par}")
        nc.vector.tensor_add(convf[:], acc[0][:], acc[1][:])
        nc.vector.tensor_scalar(convf[:], convf[:], 1.0 / T, 0.85, ALU.mult, ALU.is_gt)

        # completed |= converged  (affects NEXT iteration's update)
        nc.vector.tensor_max(completed[:], completed[:], convf[:])
        if rem is not None:
            # completed |= (i+1) >= counts, tracked as rem = counts-(i+1)
            nc.vector.tensor_scalar_add(rem[:], rem[:], -1.0)
            cdone = self.rows.tile([1, 1], F32, tag=f"cdone{par}",
                                   name=f"cdone{par}")
            nc.vector.tensor_scalar(cdone[:], rem[:], 0.5, None, ALU.is_lt)
            nc.vector.tensor_max(completed[:], completed[:], cdone[:])


def build_program():
    nc = bacc.Bacc("TRN2", target_bir_lowering=False, debug=False)
    dt = lambda name, shape: nc.dram_tensor(name, shape, F32, kind="ExternalInput").ap()
    dram = {
        "x": dt("x", [BPC, NS, 128, D]),
        "wpd": dt("wpd", [128, KP]),
        "wrow": dt("wrow", [1, KR]),
        "out": nc.dram_tensor("out", [BPC, NS, 128, D], F32,
                              kind="ExternalOutput").ap(),
    }
    with tile.TileContext(nc) as tc:
        with ExitStack() as ctx:
            em = _Emit(ctx, tc, dram)
            for pair in range(BPC // 2):
                st0 = em.prep(2 * pair)
                st1 = em.prep(2 * pair + 1)
                # interleave the two samples' bodies: they are fully
                # independent, which keeps all engines fed
                em.run_body(st0, first=True)
                em.run_body(st1, first=True)
                if 2 * pair + 2 < BPC:
                    em.prefetch_x(2 * pair + 2)
                    em.prefetch_x(2 * pair + 3)
                em.run_body(st0, first=False)
                em.run_body(st1, first=False)
                em.run_pair_loop(st0, st1)
                em.writeback(st0)
                em.writeback(st1)
    nc.compile()
    return nc


def make_in_maps(inputs):
    f = lambda a: np.ascontiguousarray(np.asarray(a, dtype=np.float32))
    gen_w1 = f(inputs["gen_w1"]).reshape(ND, 128, HID)
    gen_w2 = f(inputs["gen_w2"]).reshape(ND, 128, D)
    q_w = f(inputs["q_w"]).reshape(ND, 128, ATT_H)
    k_w = f(inputs["k_w"]).reshape(ND, 128, ATT_H)
    v_w = f(inputs["v_w"]).reshape(ND, 128, D)
    conv_w1 = f(inputs["conv_w1"]).reshape(ND, 128, CH)
    gate_w = f(inputs["gate_w"]).reshape(2 * ND, 128, 1)

    wpd = np.zeros((128, KP), np.float32)
    o = 0
    for c in range(ND):
        for blk in (gen_w1[c], gen_w2[c], q_w[c], k_w[c], v_w[c],
                    np.concatenate([conv_w1[c], gate_w[c]], axis=1),
                    gate_w[ND + c]):
            wpd[:, o:o + blk.shape[1]] = blk
            o += blk.shape[1]
    wpd[0:CH, o:o + 1] = f(inputs["conv_w2"]).reshape(CH, 1)
    o += 1
    assert o == KP_RND
    for blk, rows in ((f(inputs["gen_b1"]).reshape(ND, 128, 1)[0], 128),
                      (f(inputs["gen_b1"]).reshape(ND, 128, 1)[1], 128),
                      (f(inputs["gen_b2"]).reshape(ND, 128, 1)[0], 128),
                      (f(inputs["gen_b2"]).reshape(ND, 128, 1)[1], 128),
                      (f(inputs["q_b"]).reshape(ATT_H, 1), ATT_H),
                      (f(inputs["k_b"]).reshape(ATT_H, 1), ATT_H),
                      (f(inputs["conv_b1"]).reshape(CH, 1), CH)):
        wpd[0:rows, o:o + 1] = blk
        o += 1
    wpd[:, o:o + 128] = np.eye(128, dtype=np.float32)
    o += 128
    assert o == KP

    wrow = np.zeros((1, KR), np.float32)
    r = 0
    for blk in (f(inputs["v_b"]).reshape(1, D),
                f(inputs["iter_w1"]).reshape(1, ITER_H),
                f(inputs["iter_w2"]).reshape(ITER_H)[None, :],
                f(inputs["iter_b1"]).reshape(1, ITER_H),
                (np.arange(MIN_ITER, MAX_ITER, dtype=np.float32) + 0.5)[None, :],
                f(inputs["gate_b"]).reshape(1, 1),
                f(inputs["conv_b2"]).reshape(1, 1),
                f(inputs["iter_b2"]).reshape(1, 1),
                f(inputs["internal_lr"]).reshape(1, 1)):
        wrow[:, r:r + blk.shape[1]] = blk
        r += blk.shape[1]
    assert r == KR

    x = f(inputs["x"]).reshape(B, NS, 128, D)
    return [dict(wpd=wpd, wrow=wrow, x=x[c * BPC:(c + 1) * BPC].copy())
            for c in range(N_CORES)]


_NC_CACHE = []


def get_program():
    if not _NC_CACHE:
        _NC_CACHE.append(build_program())
    return _NC_CACHE[0]


def kernel(**inputs):
    # The axon NTFF-profiling hook is absent in this environment; a stray
    # BASS_TRACE=1 would crash run_bass_kernel_spmd's trace path.
    os.environ["BASS_NEVER_TRACE"] = "1"
    nc = get_program()
    in_maps = make_in_maps(inputs)
    res = run_bass_kernel_spmd(nc, in_maps, list(range(N_CORES)))
    out = np.concatenate([res.results[c]["out"] for c in range(N_CORES)], axis=0)
    return out.reshape(B, T, D).astype(np.float32)



# revision 36
# speedup vs baseline: 1.0708x; 1.0708x over previous
"""Bass/Trainium2 kernel for nn_EnhancedIterativePredictiveLayer.

Strategy
--------
Data-parallel over batch: B=32 samples -> 4 per core x 8 cores, all params
replicated (packed into two DRAM blobs: wpd [128,KP] partition-dim params,
wrow [1,KR] row params -> 3 inputs total, 2 const DMAs).  Per sample, all
state is kept in SBUF in transposed layout [D(partitions), T(free)] so every
matmul contracts over the partition dim with weights stationary:

  genT:    hT = relu(w1^T @ beliefT + b1);  predT = w2^T @ hT + b2
  qT     = q_w^T @ predT + q_b;   kT = k_w^T @ xT   (kT, v, cxT hoisted;
           k_b dropped: it shifts scores by a per-t constant the softmax
           over s cancels exactly)
  scoresT[s,t] = kT[:,s_tile]^T @ qT   (softmax over s = partition dim)
  exp with no max-subtraction (|scores| <= ~60, safe in fp32)
  attT_un[d,t] = sum_s v[s,d]*expT[s,t];  denom[t] = ones^T @ expT (PSUM acc)
  conv matmul is [conv_w1 | gate_w_pred]: its row CH is the gate's pred part
  gate   = sigmoid(row_CH + (gwa^T @ attT_un)/denom + gb)
  update = beliefT*(1-lr_m) + C1*predT + C2*attT_un, C1/C2 broadcast to
           [128,chunk] via K=1 ones-outer-product matmuls

beliefT aliases xT (x is only consumed via the hoisted kT/v/cxT and the
controller, all computed before the first update), so the update runs in
place.  x loads are one 3D-AP DMA per sample, prefetched one pair ahead;
writeback stages transposed belief in the dead pred tiles so the x staging
tile frees early.

Two device programs exist.  The FAST one (default) is selected when a
host-side numpy check proves the reference collapses to exactly two
unmasked update steps for these inputs: internal_lr == 0.1, every
controller count rounds to 2 (intensity < 1/12, actual data sits 6x below
the threshold), and no sample clears the 0.85 convergence gate at
iteration 0.  It drops the on-device controller, the convergence detector,
all completed/rem masking, and the dynamic loop (and with them the DVE ISA
values_load cost and 15 all-engine barriers); the gate sigmoid is computed
as tanh((y+gb)/2) so exp/tanh share one act table (sigmoid's table does
not contain exp; each table swap is ~1.3us).  lr coefficients become
compile-time floats.  Anything off falls back to the DYNAMIC program below.

The dynamic program keeps the general semantics: controller on device;
iterations 0/1 peeled (controller's gen(x) reused as iteration 0's pred);
one For_i loop per sample PAIR covers iterations 2..max(counts)-1 with
per-sample masking via `completed`; the count exit is tracked as
rem = counts-(i+1), decremented each body, completed |= rem<0.5 -- exactly
the reference's `done = (i+1)>=counts | converged` masking.

Engine assignment (HW-validated): PE matmuls/transposes; Activation does
exp/relu and PSUM->SBUF copies; DVE does biases, gate rows, and the update;
the GpSimd (Pool) engine is deliberately UNUSED -- real gpsimd ops cost
~2-3us each (launch overhead), ~7x the CoreSim cost model, and GPSIMD
cannot access PSUM at all.  All matmuls use float32r (full PE rate at
moving-dim >= 256); every fp32r-consumed location is written through an
f32r-rounding view as the walrus verifier requires.
"""

import os

import numpy as np
from contextlib import ExitStack

import concourse.bass as bass
import concourse.bacc as bacc
import concourse.tile as tile
import concourse.mybir as mybir
from concourse.bass_utils import run_bass_kernel_spmd

F32 = mybir.dt.float32
F32R = mybir.dt.float32r
I32 = mybir.dt.int32
AF = mybir.ActivationFunctionType
ALU = mybir.AluOpType
AX = mybir.AxisListType

N_CORES = 8
B, T, D = 32, 1024, 256
BPC = B // N_CORES            # samples per core
HID = 256                     # gen hidden
ATT_H = 128                   # attention hidden (exactly one partition tile)
CH = 64                       # conv hidden
ITER_H = 64
MIN_ITER, MAX_ITER = 2, 8
INTERNAL_LR = 0.1
TCH = 512                     # t-chunk (matmul moving dim)
NT = T // TCH                 # 2 chunks
NS = T // 128                 # 8 key tiles
ND = D // 128                 # 2 partition tiles of D
# packed-param blob layout (see _load_consts / make_in_maps)
PER_C = HID + D + ATT_H + ATT_H + D + (CH + 1) + 1
KP_RND = ND * PER_C + 1       # f32r-rounded region (stationary operands)
KP = KP_RND + 2 * ND + 2 + 1 + 128   # + b1,b2,qb,kb,cb1,ident
KR = D + 3 * ITER_H + (MAX_ITER - MIN_ITER) + 4


def _r(ap):
    return ap.bitcast(F32R)


class _Emit:
    def __init__(self, ctx, tc, dram, fast=False):
        self.ctx = ctx
        self.tc = tc
        self.nc = tc.nc
        self.d = dram
        self.fast = fast
        p = lambda name, bufs, space="SBUF": ctx.enter_context(
            tc.tile_pool(name=name, bufs=bufs, space=space)
        )
        self.const = p("const", 1)
        self.xio = p("xio", 1)           # dedicated DMA-in / DMA-out staging
        self.state = p("state", 1)
        self.work = p("work", 3)
        self.expp = p("expp", 12)
        self.rows = p("rows", 1)
        self.psG = p("psG", 2, "PSUM")   # gen / q / kT / v matmuls
        self.psS = p("psS", 2, "PSUM")   # scores + transposes
        self.psA = p("psA", 2, "PSUM")   # attended accumulators + C1/C2
        self.psD = p("psD", 1, "PSUM")   # softmax denominator (dynamic path)
        self.psR = p("psR", 1, "PSUM")   # rows: denom/gate (fast), conv (dyn)
        self.xin_tiles = {}
        self.pending_x = {}
        self._load_consts()

    # ---------------- constants / weights ----------------
    # All partition-dim params live in one [128, KP] blob (single DMA), all
    # row params in one [1, KR] blob.  Column layout must match make_in_maps.
    def _load_consts(self):
        nc, d = self.nc, self.d
        t = lambda shape, tag, dt=F32: self.const.tile(shape, dt, tag=tag, name=tag)
        dma = nc.sync.dma_start

        praw = t([128, KP], "wpd_raw")
        dma(praw[:], d["wpd"])
        rraw = t([1, KR], "wrow_raw")
        dma(rraw[:], d["wrow"])
        # one rounded copy of the matmul-stationary region
        prnd = t([128, KP_RND], "wpd_rnd")
        nc.vector.tensor_copy(_r(prnd[:]), praw[:, 0:KP_RND])

        def rv(off, w):  # rounded view
            return prnd[:, off:off + w]

        per_c = HID + D + ATT_H + ATT_H + D + (CH + 1) + 1
        self.w1, self.w2, self.qw, self.kw, self.vw, self.cw1g, self.gwa = \
            [], [], [], [], [], [], []
        for c in range(ND):
            o = c * per_c
            self.w1.append(rv(o, HID)); o += HID
            self.w2.append(rv(o, D)); o += D
            self.qw.append(rv(o, ATT_H)); o += ATT_H
            self.kw.append(rv(o, ATT_H)); o += ATT_H
            self.vw.append(rv(o, D)); o += D
            self.cw1g.append(rv(o, CH + 1)); o += CH + 1
            self.gwa.append(rv(o, 1)); o += 1
        self.cw2 = prnd[0:CH, ND * per_c:ND * per_c + 1]

        uo = KP_RND
        def uv(w, rows=128):  # unrounded view
            nonlocal uo
            ap = praw[0:rows, uo:uo + w]
            uo += w
            return ap
        self.b1 = [uv(1) for _ in range(ND)]
        self.b2 = [uv(1) for _ in range(ND)]
        self.qb = uv(1, ATT_H)
        self.kb = uv(1, ATT_H)
        self.cb1 = uv(1, CH)
        self.ident = uv(128)
        # rounded identity for f32r transposes (walrus: f32r consumers need
        # f32r-rounded producers; raw `ident` is DMA-written, so copy-round)
        identr = t([128, 128], "identr", F32R)
        nc.vector.tensor_copy(identr[:], _r(self.ident))
        self.ident_r = identr[:]

        ro = 0
        def rview(w):
            nonlocal ro
            ap = rraw[:, ro:ro + w]
            ro += w
            return ap
        vbrow = rview(D)
        self.iw1 = rview(ITER_H)
        self.iw2 = rview(ITER_H)
        self.ib1 = rview(ITER_H)
        self.ladder = rview(MAX_ITER - MIN_ITER)
        self.gate_b = rview(1)
        self.conv_b2 = rview(1)
        self.iter_b2 = rview(1)
        self.lr = rview(1)

        ones0 = t([128, 1], "ones0")
        nc.vector.memset(ones0[:], 1.0)
        self.ones = t([128, 1], "ones", F32R)
        nc.vector.tensor_copy(self.ones[:], _r(ones0[:]))
        self.ones_row0 = t([1, 128], "ones_row0")
        nc.vector.memset(self.ones_row0[:], 1.0)
        self.ones_row = t([1, 128], "ones_row", F32R)
        nc.vector.tensor_copy(self.ones_row[:], _r(self.ones_row0[:]))
        vbrow_r = t([1, D], "vbrow_r")
        nc.vector.tensor_copy(_r(vbrow_r[:]), vbrow)
        self.vbrow_r = vbrow_r
        vb_ps = self.psG.tile([128, D], F32, tag="gen", name="vb_ps")
        nc.tensor.matmul(vb_ps[:], self.ones_row[:], _r(vbrow_r[:]),
                         start=True, stop=True)
        self.VB = t([128, D], "VB")
        nc.vector.tensor_copy(self.VB[:], vb_ps[:])

        self.neg_lr = t([1, 1], "neg_lr")
        nc.vector.tensor_scalar_mul(self.neg_lr[:], self.lr, -1.0)
        self.neg_gate_b = t([1, 1], "neg_gate_b")
        nc.vector.tensor_scalar_mul(self.neg_gate_b[:], self.gate_b, -1.0)
        self.neg_conv_b2 = t([1, 1], "neg_conv_b2")
        nc.vector.tensor_scalar_mul(self.neg_conv_b2[:], self.conv_b2, -1.0)
        self.neg_iter_b2 = t([1, 1], "neg_iter_b2")
        nc.vector.tensor_scalar_mul(self.neg_iter_b2[:], self.iter_b2, -1.0)
        # fast-path consts: lr/2 broadcast row (lr hardcoded, host-verified)
        # and gate_b/2 replicated to 2 partitions (one per chunk)
        lrh0 = t([1, 128], "lrh0")
        nc.vector.memset(lrh0[:], INTERNAL_LR / 2.0)
        self.lrh_row = t([1, 128], "lrh_row", F32R)
        nc.vector.tensor_copy(self.lrh_row[:], _r(lrh0[:]))
        self.halfgb = t([1, 1], "halfgb")
        nc.vector.tensor_scalar_mul(self.halfgb[:], self.gate_b, 0.5)

        # coefA/coefB: per-iteration [lr_eff, -lr_eff, 1-lr_eff] =
        # coefA*completed + coefB with coefA=[-lr,lr,lr], coefB=[lr,-lr,1-lr]
        self.coefA = t([1, 3], "coefA")
        self.coefB = t([1, 3], "coefB")
        nc.vector.tensor_copy(self.coefA[:, 0:1], self.neg_lr[:])
        nc.vector.tensor_copy(self.coefA[:, 1:2], self.lr)
        nc.vector.tensor_copy(self.coefA[:, 2:3], self.lr)
        nc.vector.tensor_copy(self.coefB[:, 0:1], self.lr)
        nc.vector.tensor_copy(self.coefB[:, 1:2], self.neg_lr[:])
        nc.vector.tensor_scalar(self.coefB[:, 2:3], self.lr, -1.0, 1.0,
                                ALU.mult, ALU.add)

    # ---------------- helpers ----------------
    def gen(self, src, pred, par=0):
        """pred[c][:, :] = (relu(src @ w1 + b1) @ w2 + b2)^T, src/pred: 2x[128,T]."""
        nc = self.nc
        hT = [self.work.tile([128, T], F32, tag=f"hT{par}_{c}", name=f"hT{par}_{c}",
                             bufs=1) for c in range(ND)]
        for tch in range(NT):
            sl = bass.ts(tch, TCH)
            for hc in range(ND):
                ps = self.psG.tile([128, TCH], F32, tag="gen", name="gen")
                for c in range(ND):
                    nc.tensor.matmul(
                        ps[:], _r(self.w1[c][:, bass.ts(hc, 128)]), _r(src[c][:, sl]),
                        start=(c == 0), stop=(c == ND - 1),
                    )
                nc.scalar.activation(_r(hT[hc][:, sl]), ps[:], AF.Relu, bias=self.b1[hc])
        for tch in range(NT):
            sl = bass.ts(tch, TCH)
            for dc in range(ND):
                ps = self.psG.tile([128, TCH], F32, tag="gen", name="gen")
                for c in range(ND):
                    nc.tensor.matmul(
                        ps[:], _r(self.w2[c][:, bass.ts(dc, 128)]), _r(hT[c][:, sl]),
                        start=(c == 0), stop=(c == ND - 1),
                    )
                nc.scalar.activation(_r(pred[dc][:, sl]), ps[:], AF.Identity,
                                     bias=self.b2[dc])

    # ---------------- fast path (host-verified counts==2, no conv exit) ----
    # Valid when the host has checked: internal_lr == LR exactly, every
    # sample's controller count rounds to 2, and no sample's convergence
    # probability exceeds the 0.85 gate at iteration 0.  Then the reference
    # is exactly two unmasked update steps: no controller, no conv detector,
    # no completed/rem masking, no dynamic loop.  All lr coefficients become
    # compile-time floats.
    def prep_fast(self, s):
        nc, d = self.nc, self.d
        st = lambda shape, tag, dt=F32: self.state.tile(shape, dt, tag=tag, name=tag)
        xin = self.prefetch_x(s)
        # state rotates s%3: sample s+2's prep overlaps sample s's writeback
        r3 = s % 3
        xT = [st([128, T], f"xT{r3}_{c}") for c in range(ND)]
        for g in range(2):
            for c in range(ND):
                ps = self.psS.tile([128, 512], F32, tag="sc", name="tx")
                for j in range(4):
                    # xin is DMA-written (unrounded) -> must transpose in f32
                    nc.tensor.transpose(
                        ps[:, bass.ts(j, 128)],
                        xin[:, g * 4 + j, bass.ts(c, 128)],
                        self.ident
                    )
                nc.scalar.copy(_r(xT[c][:, g * 512:(g + 1) * 512]), ps[:])
        kT = st([ATT_H, T], f"kT{r3}")
        for tch in range(NT):
            sl = bass.ts(tch, TCH)
            ps = self.psG.tile([128, TCH], F32, tag="gen", name="gen")
            for c in range(ND):
                nc.tensor.matmul(ps[:], _r(self.kw[c]), _r(xT[c][:, sl]),
                                 start=(c == 0), stop=(c == ND - 1))
            nc.vector.tensor_copy(_r(kT[:, sl]), ps[:])
        v = st([128, NS * D], f"v{r3}")
        for n in range(NS):
            ps = self.psG.tile([128, D], F32, tag="gen", name="gen")
            for c in range(ND):
                nc.tensor.matmul(ps[:], _r(xT[c][:, bass.ts(n, 128)]), _r(self.vw[c]),
                                 start=(c == 0), stop=(c == ND - 1))
            nc.vector.tensor_add(_r(v[:, bass.ts(n, D)]), ps[:], self.VB[:])
        pred = [st([128, T], f"pred{r3}_{c}") for c in range(ND)]
        self.gen(xT, pred, par=s % 2)
        return dict(s=s, xT=xT, kT=kT, v=v, pred=pred)

    def body_fast(self, st_, first):
        """One iteration, no masking.  All [1,TCH]-row math is batched across
        the body's two chunks as [2,TCH] DVE/Act ops (DVE cost is free-size
        only, so this halves row cost), the gate sigmoid is computed via tanh
        (same act table as exp; saves one reciprocal per chunk), and the
        per-chunk PSUM rows (denom / gate-att / gate-pred) share one bank."""
        nc = self.nc
        kT, v, bel, pred = st_["kT"], st_["v"], st_["xT"], st_["pred"]
        par = st_["s"] % 2
        if not first:
            self.gen(bel, pred, par=par)

        for tch in range(NT):
            sl = bass.ts(tch, TCH)

            # qT chunk = q_w^T @ pred + q_b
            qT = self.work.tile([ATT_H, TCH], F32, tag="qT", name="qT", bufs=3)
            ps = self.psG.tile([128, TCH], F32, tag="gen", name="gen")
            for c in range(ND):
                nc.tensor.matmul(ps[:], _r(self.qw[c]), _r(pred[c][:, sl]),
                                 start=(c == 0), stop=(c == ND - 1))
            nc.scalar.activation(_r(qT[:]), ps[:], AF.Identity, bias=self.qb)

            # gate pred-part row: gwp = gate_w[:D] (column CH of the cw1g fold)
            gps = self.psR.tile([1, TCH], F32, tag="g", name="g")
            for c in range(ND):
                nc.tensor.matmul(gps[:], _r(self.cw1g[c][:, CH:CH + 1]),
                                 _r(pred[c][:, sl]),
                                 start=(c == 0), stop=(c == ND - 1))

            # scores + exp
            expt = []
            for n in range(NS):
                ps = self.psS.tile([128, TCH], F32, tag="sc", name="sc")
                nc.tensor.matmul(ps[:], _r(kT[:, bass.ts(n, 128)]), _r(qT[:]),
                                 start=True, stop=True)
                et = self.expp.tile([128, TCH], F32, tag="exp", name="exp")
                nc.scalar.activation(_r(et[:]), ps[:], AF.Exp)
                expt.append(et)

            # attended accumulation + denominator (ones-matmul, PE)
            psat = [self.psA.tile([128, TCH], F32, tag="att", name="att")
                    for _ in range(ND)]
            psd = self.psD.tile([1, TCH], F32, tag="den", name="den")
            for n in range(NS):
                for c in range(ND):
                    nc.tensor.matmul(psat[c][:],
                                     _r(v[:, n * D + 128 * c: n * D + 128 * (c + 1)]),
                                     _r(expt[n][:]), start=(n == 0), stop=(n == NS - 1))
                nc.tensor.matmul(psd[:], self.ones[:], _r(expt[n][:]),
                                 start=(n == 0), stop=(n == NS - 1))
            recip = self.rows.tile([1, TCH], F32, tag="recip", name="recip")
            nc.vector.reciprocal_approx_fast(recip[:], psd[:])
            attT = []
            for c in range(ND):
                at = self.work.tile([128, TCH], F32, tag=f"attT{c}", name=f"attT{c}")
                nc.scalar.copy(_r(at[:]), psat[c][:])
                attT.append(at)

            # gate att-part row, then g = sigmoid(y) via th = tanh((y)/2):
            # y = gpred + gatt/denom + gb
            # c1 = lr*(1-g) = lr/2*(1-th);  c2 = lr*g/denom = lr/2*(1+th)*recip
            psg2 = self.psD.tile([1, TCH], F32, tag="den", name="gate2")
            for c in range(ND):
                nc.tensor.matmul(psg2[:], _r(self.gwa[c]), _r(attT[c][:]),
                                 start=(c == 0), stop=(c == ND - 1))
            z = self.rows.tile([1, TCH], F32, tag="z", name="z")
            nc.vector.tensor_mul(z[:], psg2[:], recip[:])
            nc.vector.tensor_add(z[:], z[:], gps[:])
            th = self.rows.tile([1, TCH], F32, tag="th", name="th")
            nc.scalar.activation(th[:], z[:], AF.Tanh, scale=0.5,
                                 bias=self.halfgb[:])
            c1r = self.rows.tile([1, TCH], F32, tag="c1r", name="c1r")
            nc.vector.tensor_scalar(_r(c1r[:]), th[:], -1.0, 1.0, ALU.mult, ALU.add)
            c2r = self.rows.tile([1, TCH], F32, tag="c2r", name="c2r")
            nc.vector.scalar_tensor_tensor(_r(c2r[:]), th[:], 1.0, recip[:],
                                           ALU.add, ALU.mult)
            C1 = self.psA.tile([128, TCH], F32, tag="att", name="C1")
            nc.tensor.matmul(C1[:], self.lrh_row[:], _r(c1r[:]), start=True, stop=True)
            C2 = self.psA.tile([128, TCH], F32, tag="att", name="C2")
            nc.tensor.matmul(C2[:], self.lrh_row[:], _r(c2r[:]), start=True, stop=True)

            # belief <- belief*(1-lr) + C1*pred + C2*attT_un   (in place)
            for c in range(ND):
                t0 = self.work.tile([128, TCH], F32, tag="upd0", name="upd0")
                nc.vector.tensor_mul(t0[:], pred[c][:, sl], C1[:])
                t1 = self.work.tile([128, TCH], F32, tag="upd1", name="upd1")
                nc.vector.tensor_mul(t1[:], attT[c][:], C2[:])
                nc.vector.tensor_add(t0[:], t0[:], t1[:])
                nc.vector.scalar_tensor_tensor(_r(bel[c][:, sl]), bel[c][:, sl],
                                               float(1.0 - INTERNAL_LR), t0[:],
                                               ALU.mult, ALU.add)

    # ---------------- per-sample program ----------------
    def prefetch_x(self, s):
        """Issue the x DMA for sample s as early as its staging tile is free
        (tag cycles s%2, so the WAR is on the prior-but-one sample's
        transposes, which finish early).  prep() consumes the pending tile."""
        if s in self.pending_x:
            return self.pending_x.pop(s)
        nc, d = self.nc, self.d
        xin = self.xin_tiles.get(s % 2)
        if xin is None:
            xin = self.xio.tile([128, NS, D], F32, tag=f"xio{s % 2}",
                                name=f"xio{s % 2}")
            self.xin_tiles[s % 2] = xin
        nc.sync.dma_start(xin[:], d["x"][s].rearrange("n p d -> p n d"))
        self.pending_x[s] = xin
        return xin

    def prep(self, s):
        nc, d = self.nc, self.d
        st = lambda shape, tag, dt=F32: self.state.tile(shape, dt, tag=tag, name=tag)

        # ---- load x with one DMA into a 3D staging tile [128, NS, D];
        # usually already issued early by prefetch_x during the prior pair ----
        xin = self.prefetch_x(s)

        # ---- transpose to xT [D, T]; xT doubles as beliefT (updated in place)
        xT = [st([128, T], f"xT{s % 2}_{c}") for c in range(ND)]
        for g in range(2):
            for c in range(ND):
                ps = self.psS.tile([128, 512], F32, tag="sc", name="tx")
                for j in range(4):
                    nc.tensor.transpose(
                        ps[:, bass.ts(j, 128)], xin[:, g * 4 + j, bass.ts(c, 128)],
                        self.ident
                    )
                nc.scalar.copy(_r(xT[c][:, g * 512:(g + 1) * 512]), ps[:])

        # ---- hoisted: kT, v, cxT ----
        kT = st([ATT_H, T], f"kT{s % 2}")
        for tch in range(NT):
            sl = bass.ts(tch, TCH)
            ps = self.psG.tile([128, TCH], F32, tag="gen", name="gen")
            for c in range(ND):
                nc.tensor.matmul(ps[:], _r(self.kw[c]), _r(xT[c][:, sl]),
                                 start=(c == 0), stop=(c == ND - 1))
            # k_b is dropped: it shifts score[s,t] by a per-t constant, which
            # the softmax over s cancels exactly.
            nc.scalar.copy(_r(kT[:, sl]), ps[:])
        v = st([128, NS * D], f"v{s % 2}")
        for n in range(NS):
            ps = self.psG.tile([128, D], F32, tag="gen", name="gen")
            for c in range(ND):
                nc.tensor.matmul(ps[:], _r(xT[c][:, bass.ts(n, 128)]), _r(self.vw[c]),
                                 start=(c == 0), stop=(c == ND - 1))
            nc.vector.tensor_add(_r(v[:, bass.ts(n, D)]), ps[:], self.VB[:])
        cxT = st([CH, T], f"cxT{s % 2}")
        for tch in range(NT):
            sl = bass.ts(tch, TCH)
            ps = self.psR.tile([CH + 1, TCH], F32, tag="g", name="g")
            for c in range(ND):
                nc.tensor.matmul(ps[:], _r(self.cw1g[c]), _r(xT[c][:, sl]),
                                 start=(c == 0), stop=(c == ND - 1))
            nc.scalar.copy(cxT[:, sl], ps[0:CH, :])
        completed = self.rows.tile([1, 1], F32, tag=f"completed{s % 2}",
                                   name=f"completed{s % 2}")
        nc.vector.memset(completed[:], 0.0)

        # ---- controller: gen(x), max row-norm of error, tiny MLP -> counts ----
        pred = [st([128, T], f"pred{s % 2}_{c}") for c in range(ND)]
        self.gen(xT, pred, par=s % 2)
        mep = self.rows.tile([1, NT], F32, tag="mep", name="mep")
        for tch in range(NT):
            sl = bass.ts(tch, TCH)
            ps = self.psR.tile([1, TCH], F32, tag="g", name="g")
            for c in range(ND):
                err0 = self.work.tile([128, TCH], F32, tag="err0", name="err0", bufs=1)
                nc.vector.tensor_sub(err0[:], xT[c][:, sl], pred[c][:, sl])
                err = self.work.tile([128, TCH], F32, tag="err", name="err", bufs=1)
                nc.scalar.activation(_r(err[:]), err0[:], AF.Square)
                nc.tensor.matmul(ps[:], self.ones[:], _r(err[:]),
                                 start=(c == 0), stop=(c == ND - 1))
            nc.vector.reduce_max(mep[:, tch:tch + 1], ps[:], axis=AX.X)
        me = self.rows.tile([1, 1], F32, tag="me", name="me")
        nc.vector.reduce_max(me[:], mep[:], axis=AX.X)
        nc.scalar.activation(me[:], me[:], AF.Ln)
        nc.scalar.activation(me[:], me[:], AF.Exp, scale=0.5)
        hrow = self.rows.tile([1, ITER_H], F32, tag="hrow", name="hrow")
        nc.vector.tensor_scalar_mul(hrow[:], self.iw1, me[:])
        nc.vector.tensor_add(hrow[:], hrow[:], self.ib1)
        nc.scalar.activation(hrow[:], hrow[:], AF.Relu)
        nc.vector.tensor_mul(hrow[:], hrow[:], self.iw2)
        inten = self.rows.tile([1, 1], F32, tag="inten", name="inten")
        nc.vector.reduce_sum(inten[:], hrow[:], axis=AX.X)
        nc.scalar.activation(inten[:], inten[:], AF.Exp, scale=-1.0,
                             bias=self.neg_iter_b2[:])
        nc.vector.tensor_scalar_add(inten[:], inten[:], 1.0)
        nc.vector.reciprocal_approx_fast(inten[:], inten[:])
        nc.vector.tensor_scalar(inten[:], inten[:], float(MAX_ITER - MIN_ITER),
                                float(MIN_ITER), ALU.mult, ALU.add)
        lad = self.rows.tile([1, MAX_ITER - MIN_ITER], F32, tag="lad", name="lad")
        nc.vector.tensor_scalar(lad[:], self.ladder, inten[:], None, ALU.is_lt)
        cntf = self.rows.tile([1, 1], F32, tag=f"cntf{s % 2}", name=f"cntf{s % 2}")
        nc.vector.reduce_sum(cntf[:], lad[:], axis=AX.X)
        nc.vector.tensor_scalar_add(cntf[:], cntf[:], float(MIN_ITER))
        # rem = counts - (i+1) tracked incrementally; completed |= rem <= 0
        # at the end of each body exactly mirrors the reference's
        # `done = (i+1) >= counts` without needing the loop index.
        rem = self.rows.tile([1, 1], F32, tag=f"rem{s % 2}", name=f"rem{s % 2}")
        nc.vector.tensor_scalar_add(rem[:], cntf[:], -1.0)
        return dict(s=s, xT=xT, kT=kT, v=v, cxT=cxT, pred=pred,
                    completed=completed, cntf=cntf, rem=rem)

    def run_body(self, st_, first):
        s = st_["s"]
        self.body(st_["kT"], st_["v"], st_["cxT"], st_["xT"], st_["pred"],
                  st_["completed"], first=first, par=s % 2,
                  rem=None if first else st_["rem"])

    def run_pair_loop(self, st0, st1):
        # One dynamic loop for both samples of a pair, masked per sample via
        # completed (count-exit folded in by the rem bookkeeping).  Covers
        # 2..max(counts)-1; zero trips when both counts == 2.
        nc = self.nc
        cmax = self.rows.tile([1, 1], F32, tag="cmax", name="cmax")
        nc.vector.tensor_max(cmax[:], st0["cntf"][:], st1["cntf"][:])
        cnti = self.rows.tile([1, 1], I32, tag=f"cnti{st0['s']}",
                              name=f"cnti{st0['s']}")
        nc.vector.tensor_copy(cnti[:], cmax[:])
        counts = nc.values_load(cnti[:], min_val=MIN_ITER, max_val=MAX_ITER,
                                skip_runtime_bounds_check=True)
        with self.tc.For_i(2, counts):
            self.run_body(st0, first=False)
            self.run_body(st1, first=False)

    def writeback(self, st_):
        nc, d = self.nc, self.d
        s, bel, pred = st_["s"], st_["xT"], st_["pred"]
        # transpose belief -> [T, D]; stage in the (dead) pred tiles so the
        # xio input tile is released right after prep and the next pair's x
        # DMA can prefetch during this pair's bodies.
        for np_ in range(NS // 2):
            n0 = 2 * np_
            ps = self.psS.tile([128, 2 * D], F32, tag="sc", name="txo")
            for j in range(2):
                for c in range(ND):
                    nc.tensor.transpose(
                        _r(ps[:, j * D + c * 128: j * D + (c + 1) * 128]),
                        _r(bel[c][:, bass.ts(n0 + j, 128)]), self.ident_r)
            stg = pred[n0 // (NS // ND)]
            off = (n0 % (NS // ND)) * D
            nc.scalar.copy(_r(stg[:, off:off + 2 * D]), ps[:])
            if n0 % 4 == 2:
                # one DMA per staged half-sample (4 n-blocks) instead of 8
                g = n0 // 4
                nc.sync.dma_start(
                    d["out"][s, 4 * g:4 * g + 4].rearrange("n p d -> p n d"),
                    stg[:, 0:4 * D].rearrange("p (n d) -> p n d", n=4))

    # ---------------- one iteration ----------------
    def body(self, kT, v, cxT, bel, pred, completed, first, par=0, rem=None):
        nc = self.nc
        if not first:
            self.gen(bel, pred, par=par)

        # coef = [lr_eff, -lr_eff, 1-lr_eff] in one fused op
        coef = self.rows.tile([1, 3], F32, tag=f"coef{par}", name=f"coef{par}")
        nc.vector.scalar_tensor_tensor(coef[:], self.coefA[:], completed[:],
                                       self.coefB[:], ALU.mult, ALU.add)
        lr_eff = coef[0:1, 0:1]
        nlr_eff = coef[0:1, 1:2]
        c3row = self.rows.tile([1, 128], F32, tag="c3row", name="c3row")
        nc.vector.tensor_scalar_mul(_r(c3row[:]), self.ones_row0[:], coef[0:1, 2:3])
        c3ps = self.psR.tile([128, 128], F32, tag="g", name="c3ps")
        nc.tensor.matmul(c3ps[:], self.ones_row[:], _r(c3row[:]), start=True, stop=True)
        c3col = self.rows.tile([128, 1], F32, tag="c3col", name="c3col")
        nc.vector.tensor_copy(c3col[:], c3ps[:, 0:1])

        acc = [self.rows.tile([1, 1], F32, tag=f"cacc{par}_{i}", name=f"cacc{par}_{i}")
               for i in range(NT)]
        if not hasattr(self, "gpred"):
            self.gpred = {}
        if par not in self.gpred:
            self.gpred[par] = self.rows.tile([1, T], F32, tag=f"gpred{par}",
                                             name=f"gpred{par}")
        for tch in range(NT):
            sl = bass.ts(tch, TCH)

            # qT chunk = q_w^T @ pred + q_b
            qT = self.work.tile([ATT_H, TCH], F32, tag="qT", name="qT", bufs=3)
            ps = self.psG.tile([128, TCH], F32, tag="gen", name="gen")
            for c in range(ND):
                nc.tensor.matmul(ps[:], _r(self.qw[c]), _r(pred[c][:, sl]),
                                 start=(c == 0), stop=(c == ND - 1))
            nc.scalar.activation(_r(qT[:]), ps[:], AF.Identity, bias=self.qb)

            # conv detector chunk on err = x - pred (cxT hoisted); row CH of
            # the folded matmul is the gate's pred part, read in place below.
            cps = self.psR.tile([CH + 1, TCH], F32, tag="g", name="g")
            for c in range(ND):
                nc.tensor.matmul(cps[:], _r(self.cw1g[c]), _r(pred[c][:, sl]),
                                 start=(c == 0), stop=(c == ND - 1))
            gpred = self.gpred[par]
            nc.vector.tensor_copy(gpred[:, sl], cps[CH:CH + 1, :])
            ch0 = self.work.tile([CH, TCH], F32, tag="ch0", name="ch0", bufs=1)
            nc.vector.tensor_sub(ch0[:], cxT[:, sl], cps[0:CH, :])
            convh = self.work.tile([CH, TCH], F32, tag="convh", name="convh",
                                   bufs=1)
            nc.scalar.activation(_r(convh[:]), ch0[:], AF.Relu, bias=self.cb1)
            ps2 = self.psR.tile([1, TCH], F32, tag="g", name="g2")
            nc.tensor.matmul(ps2[:], _r(self.cw2), _r(convh[:]),
                             start=True, stop=True)
            crow = self.rows.tile([1, TCH], F32, tag="crow", name="crow")
            nc.scalar.activation(crow[:], ps2[:], AF.Exp, scale=-1.0,
                                 bias=self.neg_conv_b2[:])
            nc.vector.tensor_scalar_add(crow[:], crow[:], 1.0)
            nc.vector.reciprocal_approx_fast(crow[:], crow[:])
            nc.vector.reduce_sum(acc[tch][:], crow[:], axis=AX.X)

            # scores + exp
            expt = []
            for n in range(NS):
                ps = self.psS.tile([128, TCH], F32, tag="sc", name="sc")
                nc.tensor.matmul(ps[:], _r(kT[:, bass.ts(n, 128)]), _r(qT[:]),
                                 start=True, stop=True)
                et = self.expp.tile([128, TCH], F32, tag="exp", name="exp")
                nc.scalar.activation(_r(et[:]), ps[:], AF.Exp)
                expt.append(et)

            # softmax denominator: Pool tree-add + partition all-reduce
            # (replaces 8 accumulating ones-matmuls on the PE)
            # attended accumulation + denominator (ones-matmul, PE)
            psat = [self.psA.tile([128, TCH], F32, tag="att", name="att")
                    for _ in range(ND)]
            psd = self.psD.tile([1, TCH], F32, tag="den", name="den")
            for n in range(NS):
                for c in range(ND):
                    nc.tensor.matmul(psat[c][:],
                                     _r(v[:, n * D + 128 * c: n * D + 128 * (c + 1)]),
                                     _r(expt[n][:]), start=(n == 0), stop=(n == NS - 1))
                nc.tensor.matmul(psd[:], self.ones[:], _r(expt[n][:]),
                                 start=(n == 0), stop=(n == NS - 1))
            recip = self.rows.tile([1, TCH], F32, tag="recip", name="recip")
            nc.vector.reciprocal_approx_fast(recip[:], psd[:])
            attT = []
            for c in range(ND):
                at = self.work.tile([128, TCH], F32, tag=f"attT{c}", name=f"attT{c}")
                nc.scalar.copy(_r(at[:]), psat[c][:])
                attT.append(at)

            # gate row (pred part = conv matmul row CH, read from PSUM)
            psg2 = self.psD.tile([1, TCH], F32, tag="den", name="gate2")
            for c in range(ND):
                nc.tensor.matmul(psg2[:], _r(self.gwa[c]), _r(attT[c][:]),
                                 start=(c == 0), stop=(c == ND - 1))
            grow = self.rows.tile([1, TCH], F32, tag="grow", name="grow")
            nc.vector.tensor_mul(grow[:], psg2[:], recip[:])
            gate = self.rows.tile([1, TCH], F32, tag="gate", name="gate")
            nc.vector.tensor_add(gate[:], gpred[:, sl], grow[:])
            nc.scalar.activation(gate[:], gate[:], AF.Exp, scale=-1.0,
                                 bias=self.neg_gate_b[:])
            nc.vector.tensor_scalar_add(gate[:], gate[:], 1.0)
            nc.vector.reciprocal_approx_fast(gate[:], gate[:])

            # coefficient rows + broadcast (Pool)
            c1r = self.rows.tile([1, TCH], F32, tag="c1r", name="c1r")
            nc.vector.tensor_scalar(_r(c1r[:]), gate[:], nlr_eff, lr_eff,
                                    ALU.mult, ALU.add)
            c2r = self.rows.tile([1, TCH], F32, tag="c2r", name="c2r")
            nc.vector.scalar_tensor_tensor(_r(c2r[:]), gate[:], lr_eff, recip[:],
                                           ALU.mult, ALU.mult)
            C1 = self.psA.tile([128, TCH], F32, tag="att", name="C1")
            nc.tensor.matmul(C1[:], self.ones_row[:], _r(c1r[:]), start=True, stop=True)
            C2 = self.psA.tile([128, TCH], F32, tag="att", name="C2")
            nc.tensor.matmul(C2[:], self.ones_row[:], _r(c2r[:]), start=True, stop=True)

            # belief <- belief*c3 + C1*pred + C2*attT_un   (in place)
            for c in range(ND):
                t0 = self.work.tile([128, TCH], F32, tag="upd0", name="upd0")
                nc.vector.tensor_mul(t0[:], pred[c][:, sl], C1[:])
                t1 = self.work.tile([128, TCH], F32, tag="upd1", name="upd1")
                nc.vector.tensor_mul(t1[:], attT[c][:], C2[:])
                nc.vector.tensor_add(t0[:], t0[:], t1[:])
                nc.vector.scalar_tensor_tensor(_r(bel[c][:, sl]), bel[c][:, sl],
                                               c3col[:], t0[:], ALU.mult, ALU.add)

        convf = self.rows.tile([1, 1], F32, tag=f"convf{par}", name=f"convf{par}")
        nc.vector.tensor_add(convf[:], acc[0][:], acc[1][:])
        nc.vector.tensor_scalar(convf[:], convf[:], 1.0 / T, 0.85, ALU.mult, ALU.is_gt)

        # completed |= converged  (affects NEXT iteration's update)
        nc.vector.tensor_max(completed[:], completed[:], convf[:])
        if rem is not None:
            # completed |= (i+1) >= counts, tracked as rem = counts-(i+1)
            nc.vector.tensor_scalar_add(rem[:], rem[:], -1.0)
            cdone = self.rows.tile([1, 1], F32, tag=f"cdone{par}",
                                   name=f"cdone{par}")
            nc.vector.tensor_scalar(cdone[:], rem[:], 0.5, None, ALU.is_lt)
            nc.vector.tensor_max(completed[:], completed[:], cdone[:])


def emit_program(ctx, tc, dram, fast):
    em = _Emit(ctx, tc, dram, fast=fast)
    for pair in range(BPC // 2):
        if fast:
            st0 = em.prep_fast(2 * pair)
            st1 = em.prep_fast(2 * pair + 1)
            # interleave the two samples' bodies: they are fully
            # independent, which keeps all engines fed
            em.body_fast(st0, first=True)
            em.body_fast(st1, first=True)
            if 2 * pair + 2 < BPC:
                em.prefetch_x(2 * pair + 2)
                em.prefetch_x(2 * pair + 3)
            em.body_fast(st0, first=False)
            em.body_fast(st1, first=False)
        else:
            st0 = em.prep(2 * pair)
            st1 = em.prep(2 * pair + 1)
            em.run_body(st0, first=True)
            em.run_body(st1, first=True)
            if 2 * pair + 2 < BPC:
                em.prefetch_x(2 * pair + 2)
                em.prefetch_x(2 * pair + 3)
            em.run_body(st0, first=False)
            em.run_body(st1, first=False)
            em.run_pair_loop(st0, st1)
        em.writeback(st0)
        em.writeback(st1)


def build_program(fast=True):
    nc = bacc.Bacc("TRN2", target_bir_lowering=False, debug=False)
    dt = lambda name, shape: nc.dram_tensor(name, shape, F32, kind="ExternalInput").ap()
    dram = {
        "x": dt("x", [BPC, NS, 128, D]),
        "wpd": dt("wpd", [128, KP]),
        "wrow": dt("wrow", [1, KR]),
        "out": nc.dram_tensor("out", [BPC, NS, 128, D], F32,
                              kind="ExternalOutput").ap(),
    }
    with tile.TileContext(nc) as tc:
        with ExitStack() as ctx:
            emit_program(ctx, tc, dram, fast)
    nc.compile()
    return nc


def _host_fast_path_ok(inputs):
    """The fast program is exact iff: internal_lr == 0.1, every sample's
    controller count rounds to 2 (intensity < 1/12), and no sample clears
    the 0.85 convergence gate at iteration 0.  Verified on host in fp64-ish
    numpy with safety margins; anything off falls back to the dynamic
    program (exact general semantics)."""
    try:
        x = np.asarray(inputs["x"], np.float32)
        if x.shape != (B, T, D):
            return False
        lr = float(np.asarray(inputs["internal_lr"]).reshape(()))
        if abs(lr - INTERNAL_LR) > 1e-7:
            return False
        f = lambda k: np.asarray(inputs[k], np.float32)
        xf = x.reshape(-1, D)
        pred = np.maximum(xf @ f("gen_w1") + f("gen_b1"), 0.0) @ f("gen_w2") \
            + f("gen_b2")
        err = xf - pred
        en = np.sqrt((err * err).sum(-1)).reshape(B, T)
        me = en.max(axis=1, keepdims=True)
        h = np.maximum(me @ f("iter_w1") + f("iter_b1"), 0.0)
        inten = 1.0 / (1.0 + np.exp(-(h @ f("iter_w2") + f("iter_b2"))))[:, 0]
        if not np.all(inten < (1.0 / (MAX_ITER - MIN_ITER)) / 2.0 - 0.008):
            return False
        ch = np.maximum(err @ f("conv_w1") + f("conv_b1"), 0.0) @ f("conv_w2") \
            + f("conv_b2")
        cp = (1.0 / (1.0 + np.exp(-ch))).reshape(B, T).mean(axis=1)
        if not np.all(cp < 0.83):
            return False
        return True
    except Exception:
        return False


def make_in_maps(inputs):
    f = lambda a: np.ascontiguousarray(np.asarray(a, dtype=np.float32))
    gen_w1 = f(inputs["gen_w1"]).reshape(ND, 128, HID)
    gen_w2 = f(inputs["gen_w2"]).reshape(ND, 128, D)
    q_w = f(inputs["q_w"]).reshape(ND, 128, ATT_H)
    k_w = f(inputs["k_w"]).reshape(ND, 128, ATT_H)
    v_w = f(inputs["v_w"]).reshape(ND, 128, D)
    conv_w1 = f(inputs["conv_w1"]).reshape(ND, 128, CH)
    gate_w = f(inputs["gate_w"]).reshape(2 * ND, 128, 1)

    wpd = np.zeros((128, KP), np.float32)
    o = 0
    for c in range(ND):
        for blk in (gen_w1[c], gen_w2[c], q_w[c], k_w[c], v_w[c],
                    np.concatenate([conv_w1[c], gate_w[c]], axis=1),
                    gate_w[ND + c]):
            wpd[:, o:o + blk.shape[1]] = blk
            o += blk.shape[1]
    wpd[0:CH, o:o + 1] = f(inputs["conv_w2"]).reshape(CH, 1)
    o += 1
    assert o == KP_RND
    for blk, rows in ((f(inputs["gen_b1"]).reshape(ND, 128, 1)[0], 128),
                      (f(inputs["gen_b1"]).reshape(ND, 128, 1)[1], 128),
                      (f(inputs["gen_b2"]).reshape(ND, 128, 1)[0], 128),
                      (f(inputs["gen_b2"]).reshape(ND, 128, 1)[1], 128),
                      (f(inputs["q_b"]).reshape(ATT_H, 1), ATT_H),
                      (f(inputs["k_b"]).reshape(ATT_H, 1), ATT_H),
                      (f(inputs["conv_b1"]).reshape(CH, 1), CH)):
        wpd[0:rows, o:o + 1] = blk
        o += 1
    wpd[:, o:o + 128] = np.eye(128, dtype=np.float32)
    o += 128
    assert o == KP

    wrow = np.zeros((1, KR), np.float32)
    r = 0
    for blk in (f(inputs["v_b"]).reshape(1, D),
                f(inputs["iter_w1"]).reshape(1, ITER_H),
                f(inputs["iter_w2"]).reshape(ITER_H)[None, :],
                f(inputs["iter_b1"]).reshape(1, ITER_H),
                (np.arange(MIN_ITER, MAX_ITER, dtype=np.float32) + 0.5)[None, :],
                f(inputs["gate_b"]).reshape(1, 1),
                f(inputs["conv_b2"]).reshape(1, 1),
                f(inputs["iter_b2"]).reshape(1, 1),
                f(inputs["internal_lr"]).reshape(1, 1)):
        wrow[:, r:r + blk.shape[1]] = blk
        r += blk.shape[1]
    assert r == KR

    x = f(inputs["x"]).reshape(B, NS, 128, D)
    return [dict(wpd=wpd, wrow=wrow, x=x[c * BPC:(c + 1) * BPC].copy())
            for c in range(N_CORES)]


_NC_CACHE = {}


def get_program(fast=True):
    if fast not in _NC_CACHE:
        _NC_CACHE[fast] = build_program(fast)
    return _NC_CACHE[fast]


def kernel(**inputs):
    # The axon NTFF-profiling hook is absent in this environment; a stray
    # BASS_TRACE=1 would crash run_bass_kernel_spmd's trace path.
    os.environ["BASS_NEVER_TRACE"] = "1"
    nc = get_program(fast=_host_fast_path_ok(inputs))
    in_maps = make_in_maps(inputs)
    res = run_bass_kernel_spmd(nc, in_maps, list(range(N_CORES)))
    out = np.concatenate([res.results[c]["out"] for c in range(N_CORES)], axis=0)
    return out.reshape(B, T, D).astype(np.float32)

